# revision 5
# baseline (speedup 1.0000x reference)
"""BioMech feature extraction on Trainium2: 8 NeuronCores, pure data-parallel.

Self-contained: takes full inputs foot/shank/thigh [8192, 12, 256] fp32,
returns [8192, 44] fp32 feature matrix matching the reference stack order.

V1 design (vs. earlier baseline):
  - Transposed DFT: stationary = DFT weight k-tiles [t, k], moving = fz_T
    [t, s] -> F[k, s] in PSUM.  |F|^2 via one ACT Square (PSUM->SBUF fp16),
    then tot/scn/hf all come from a single pair of weighted-sum matmuls
    (moving = [ones | k | hf-mask]) landing [s, 3] sample-major in PSUM.
    Kills the per-side ACT square+accum calls, the hf DVE reduce, the
    sqrt(k) DFT columns, and the muN extraction.
  - Peak tree: first level is tensor_tensor(abs_max) straight off the raw
    fp16 block, killing the full-width |x| mask pass.  Small |fz| mask kept
    for the impact feature only.
  - Sum x and sum x^2 for fz via ones-matmuls over XT / XT2 (exact, replaces
    the Parseval reconstruction).
  - GSQ (gyro squares for Sxx) split three ways: 1/3 DVE tensor_tensor,
    1/3 ACT Square, 1/3 GPSIMD tensor_tensor (the idle engine).
  - Everything else (impact/zcr/vib ACT sign/abs accums, bn_stats shank
    pair, gyro Sx/Sxx ones-matmuls, powers for moments) as in baseline.
"""

import contextlib

import numpy as np

import concourse.bacc as bacc
import concourse.bass as _bass
import concourse.tile as tile
import concourse.mybir as mybir
from concourse.bass_utils import run_bass_kernel_spmd

F32 = mybir.dt.float32
F16 = mybir.dt.float16
I16 = mybir.dt.int16
AF = mybir.ActivationFunctionType
ALU = mybir.AluOpType
AX = mybir.AxisListType

N_CORES = 8
B_FULL = 8192
T = 256
P = 128
BC = B_FULL // N_CORES          # 1024 samples per core
NBLK = BC // P                  # 8 blocks
NCH = 22
NBINS = 129
HF_BIN = 60
NK = 256                        # transposed-DFT k columns (2 tiles of 128)
EPS = 1e-6
NS = NBLK * 2                   # (block, side) stat slots

# packed channel order:
# 0 fzL, 1 fzR, 2 szL, 3 szR, 4:7 fgL, 7:10 fgR, 10:13 sgL, 13:16 sgR,
# 16:19 tgL, 19:22 tgR
SRC = [("foot", 2), ("foot", 8), ("shank", 2), ("shank", 8),
       ("foot", 3), ("foot", 4), ("foot", 5), ("foot", 9), ("foot", 10), ("foot", 11),
       ("shank", 3), ("shank", 4), ("shank", 5), ("shank", 9), ("shank", 10), ("shank", 11),
       ("thigh", 3), ("thigh", 4), ("thigh", 5), ("thigh", 9), ("thigh", 10), ("thigh", 11)]

NG = 18                         # gyro channels (packed 4..21)


def build_consts():
    t = np.arange(T, dtype=np.float64)
    # transposed-DFT weights: W[ck, trow, j] = basis_j(ck*128 + trow)
    # cols: tile0 = C_0..C_127; tile1 = [C_128, S_1..S_127]
    k_c = np.arange(128, dtype=np.float64)              # tile0 cos ks
    ang_c = 2.0 * np.pi * np.outer(t, k_c) / T
    tile0 = np.cos(ang_c)                               # [256, 128]
    k_s = np.arange(1, 128, dtype=np.float64)
    ang_s = 2.0 * np.pi * np.outer(t, k_s) / T
    c128 = np.cos(np.pi * t)[:, None]                   # (-1)^t
    tile1 = np.concatenate([c128, np.sin(ang_s)], axis=1)   # [256, 128]
    w = np.concatenate([tile0, tile1], axis=1)          # [256, 256]
    w = np.ascontiguousarray(w.reshape(2, P, NK), dtype=np.float16)

    # weighted-sum moving vectors per k-tile: [ones | k | hf-mask]
    wv = np.zeros((2, P, 3), dtype=np.float16)
    wv[0, :, 0] = 1.0
    wv[0, :, 1] = np.arange(128)
    wv[0, :, 2] = (np.arange(128) >= HF_BIN)
    wv[1, :, 0] = 1.0
    wv[1, 0, 1] = 128.0
    wv[1, 1:, 1] = np.arange(1, 128)
    wv[1, 0, 2] = 1.0
    wv[1, 1:, 2] = (np.arange(1, 128) >= HF_BIN)

    ones = np.ones((P, 1), dtype=np.float16)
    return {"w": w, "wv": wv, "ones": ones}


def build_nc():
    nc = bacc.Bacc("TRN2", target_bir_lowering=False, debug=False,
                   num_devices=N_CORES)
    x_d = nc.dram_tensor("x", [BC, NCH, T], F16, kind="ExternalInput")
    xt_d = nc.dram_tensor("xt", [NBLK, P, 2, 2, P], F16, kind="ExternalInput")
    g8_d = nc.dram_tensor("g8", [NBLK, P, 2, NG, P], F16,
                          kind="ExternalInput")
    w_d = nc.dram_tensor("w", [2, P, NK], F16, kind="ExternalInput")
    wv_d = nc.dram_tensor("wv", [2, P, 3], F16, kind="ExternalInput")
    on_d = nc.dram_tensor("ones", [P, 1], F16, kind="ExternalInput")
    out_d = nc.dram_tensor("out", [BC, 44], F32, kind="ExternalOutput")

    with tile.TileContext(nc) as tc:
        _body(tc, x_d, xt_d, g8_d, w_d, wv_d, on_d, out_d)
    nc.compile()
    return nc


def _body(tc, x_d, xt_d, g8_d, w_d, wv_d, on_d, out_d):
    nc = tc.nc
    ctx = contextlib.ExitStack()
    with ctx:
        pers = ctx.enter_context(tc.tile_pool(name="pers", bufs=1))
        p_in = ctx.enter_context(tc.tile_pool(name="xin", bufs=3))
        p_xt = ctx.enter_context(tc.tile_pool(name="xtin", bufs=3))
        p_g8 = ctx.enter_context(tc.tile_pool(name="g8in", bufs=3))
        p_gsq = ctx.enter_context(tc.tile_pool(name="gsq", bufs=2))
        p_abs = ctx.enter_context(tc.tile_pool(name="abs", bufs=2))
        p_tree = ctx.enter_context(tc.tile_pool(name="tree", bufs=2))
        p_pow = ctx.enter_context(tc.tile_pool(name="pow", bufs=2))
        p_p2 = ctx.enter_context(tc.tile_pool(name="p2", bufs=2))
        p_scr = ctx.enter_context(tc.tile_pool(name="scr", bufs=2))
        p_junk = ctx.enter_context(tc.tile_pool(name="junk", bufs=4))
        p_small = ctx.enter_context(tc.tile_pool(name="small", bufs=3))
        p_psf = ctx.enter_context(tc.tile_pool(name="psf", bufs=2, space="PSUM"))
        p_psw = ctx.enter_context(tc.tile_pool(name="psw", bufs=2, space="PSUM"))
        p_psk = ctx.enter_context(tc.tile_pool(name="psk", bufs=2, space="PSUM"))
        fin = ctx.enter_context(tc.tile_pool(name="fin", bufs=1))

        V = nc.vector
        SC = nc.scalar
        G = nc.gpsimd
        TE = nc.tensor

        # ---- constants ----
        w_sb = pers.tile([P, 2, NK], F16, tag="w", name="w_sb")
        wv_sb = pers.tile([P, 2, 3], F16, tag="wv", name="wv_sb")
        on_sb = pers.tile([P, 1], F16, tag="ones", name="on_sb")
        nc.sync.dma_start(out=w_sb[:], in_=w_d.ap().rearrange("j p n -> p j n"))
        nc.sync.dma_start(out=wv_sb[:], in_=wv_d.ap().rearrange("j p n -> p j n"))
        nc.sync.dma_start(out=on_sb[:], in_=on_d.ap())

        def stat(tag, n=NS):
            return pers.tile([P, n], F32, tag=tag, name=tag)

        # thw[:, b, side, 0:3] = [tot, scn, hf]
        thw_s = pers.tile([P, NBLK, 2, 3], F32, tag="thw", name="thw")
        imp_s = stat("imp")     # sum sign(|fz| - 0.3pk)
        zc_s = stat("zc")       # sum sign(x_t * x_{t+1})
        vib_s = stat("vib")     # sum |diff sz|
        # psK cols: 0,1 habs0 (L,R); 2,3 habs1 (L,R); 4,5 Sx3; 6,7 Sx4;
        # 8:26 gyro Sx; 26:44 gyro Sxx; 44,45 SxF (L,R); 46,47 sqF (L,R)
        mg_s = pers.tile([P, NBLK, 48], F32, tag="mg", name="mg_s")
        bnS_s = pers.tile([P, NBLK, 6], F32, tag="bnS", name="bnS")
        out_t = pers.tile([P, NBLK, 44], F32, tag="out", name="out_t")

        # preload the Sqrt/Ln ACT table sets during the DMA-fill startup so
        # the final phase doesn't pay the table-load latency.
        warm = pers.tile([P, 2], F32, tag="warm", name="warm")
        nc.vector.memset(warm[:], 1.0)
        SC.activation(warm[:, 0:1], warm[:, 1:2], AF.Sqrt)
        SC.activation(warm[:, 0:1], warm[:, 1:2], AF.Ln)

        x_ap = x_d.ap()
        xt_ap = xt_d.ap()
        g8_ap = g8_d.ap()

        # ============ final batched scalar phase (block range) ============
        def emit_final(b0, b1):
            bs = slice(b0, b1)

            def v2(tbl):
                return tbl[:].rearrange("p (b s) -> p b s", s=2)[:, bs, :]

            def ft(tag, shape=(P, NBLK, 2)):
                t = fin.tile(list(shape), F32, tag=tag, name=tag)
                return t[:, bs]

            # ratio = log1p(f_pk / (s_pk + 1e-4))  [Ln later]
            r_spk = ft("r_spk")
            V.tensor_scalar(r_spk[:], out_t[:, bs, 2:4], 1e-4, None, op0=ALU.add)
            V.reciprocal(r_spk[:], r_spk[:])
            ratio_arg = ft("ratio_arg")
            V.tensor_mul(ratio_arg[:], out_t[:, bs, 0:2], r_spk[:])

            # total power reciprocal (shared by hf and sc)
            r_tot = ft("r_tot")
            V.tensor_scalar(r_tot[:], thw_s[:, bs, :, 0], EPS, None, op0=ALU.add)
            V.reciprocal(r_tot[:], r_tot[:])
            V.tensor_mul(out_t[:, bs, 6:8], thw_s[:, bs, :, 2], r_tot[:])
            V.scalar_tensor_tensor(out_t[:, bs, 16:18], thw_s[:, bs, :, 1],
                               1.0 / NBINS, r_tot[:],
                               op0=ALU.mult, op1=ALU.mult)

            # dur, vib, zcr scaled counts
            V.tensor_scalar(out_t[:, bs, 18:20], v2(imp_s), 1.0 / (2 * T), 0.5,
                        op0=ALU.mult, op1=ALU.add)
            V.tensor_scalar(out_t[:, bs, 12:14], v2(vib_s), 1.0 / (T - 1), None,
                        op0=ALU.mult)
            V.tensor_scalar(out_t[:, bs, 42:44], v2(zc_s), -1.0 / (2 * (T - 1)),
                        0.5, op0=ALU.mult, op1=ALU.add)

            # decay = h0 / (h1 + 128e-6)
            hv = mg_s[:, bs, 0:4].rearrange("p b (h s) -> p b h s", h=2)
            dden = ft("dden")
            V.tensor_scalar(dden[:], hv[:, :, 1, :], (T // 2) * EPS, None,
                        op0=ALU.add)
            V.reciprocal(dden[:], dden[:])
            V.tensor_mul(out_t[:, bs, 10:12], hv[:, :, 0, :], dden[:])

            # asym_acc = |f_pk - s_pk|
            aa = ft("aa")
            V.tensor_sub(aa[:], out_t[:, bs, 0:2], out_t[:, bs, 2:4])
            SC.activation(out_t[:, bs, 32:34], aa[:], AF.Abs)

            # fz sums: mu = SxF/T; sqF = sum x^2; m2F = sqF - SxF^2/T
            SxF = mg_s[:, bs, 44:46]
            sqF = mg_s[:, bs, 46:48]
            mu = ft("mu")
            V.tensor_scalar(mu[:], SxF, 1.0 / T, None, op0=ALU.mult)
            qF = ft("qF")                 # mu^2
            V.tensor_mul(qF[:], mu[:], mu[:])
            m2F = ft("m2F")               # sum (x-mu)^2
            V.scalar_tensor_tensor(m2F[:], qF[:], -float(T), sqF,
                               op0=ALU.mult, op1=ALU.add)

            # central moments from raw sums:
            # m3c = Sx3 - 3 mu sqF + 2 T mu^3
            # m4c = Sx4 - 4 mu Sx3 + 6 mu^2 sqF - 3 T mu^4
            Sx3 = mg_s[:, bs, 4:6]
            Sx4 = mg_s[:, bs, 6:8]
            mu2 = qF
            muSq = ft("muSq")                       # mu * sqF
            V.tensor_mul(muSq[:], mu[:], sqF)
            mu3 = ft("mu3")
            V.tensor_mul(mu3[:], mu2[:], mu[:])
            t1 = ft("t1")                           # Sx3 - 3 mu sqF
            V.scalar_tensor_tensor(t1[:], muSq[:], -3.0, Sx3,
                               op0=ALU.mult, op1=ALU.add)
            m3c = ft("m3c")
            V.scalar_tensor_tensor(m3c[:], mu3[:], 2.0 * T, t1[:],
                               op0=ALU.mult, op1=ALU.add)
            muSx3 = ft("muSx3")
            V.tensor_mul(muSx3[:], mu[:], Sx3)
            mu2sq = ft("mu2sq")                     # mu^2 * sqF
            V.tensor_mul(mu2sq[:], mu2[:], sqF)
            mu4 = ft("mu4")
            V.tensor_mul(mu4[:], mu2[:], mu2[:])
            t2 = ft("t2")                           # Sx4 - 4 mu Sx3
            V.scalar_tensor_tensor(t2[:], muSx3[:], -4.0, Sx4,
                               op0=ALU.mult, op1=ALU.add)
            t3 = ft("t3")                           # + 6 mu^2 sqF
            V.scalar_tensor_tensor(t3[:], mu2sq[:], 6.0, t2[:],
                               op0=ALU.mult, op1=ALU.add)
            m4c = ft("m4c")
            V.scalar_tensor_tensor(m4c[:], mu4[:], -3.0 * T, t3[:],
                               op0=ALU.mult, op1=ALU.add)

            # shank stats from bn call 0: even = szL, odd = szR
            meS = bnS_s[:, bs, 1:5:3]               # [p, b, 2] means (cols 1,4)
            M2S = bnS_s[:, bs, 2:6:3]               # [p, b, 2] sum (y-mu)^2 (cols 2,5)
            sqS = ft("sqS")                         # sum y^2 = M2 + T mu^2
            uS = ft("uS")
            V.tensor_mul(uS[:], meS, meS)
            V.scalar_tensor_tensor(sqS[:], uS[:], float(T), M2S,
                               op0=ALU.mult, op1=ALU.add)

            # var_ratio = log1p(m2F / (M2S + 255e-4))  [Ln later]
            vr = ft("vr")
            V.tensor_scalar(vr[:], M2S, (T - 1) * 1e-4, None, op0=ALU.add)
            V.reciprocal(vr[:], vr[:])
            vra = ft("vra")
            V.tensor_mul(vra[:], m2F[:], vr[:])

            # gyro M2 group sums from the PE Sx/Sxx stats:
            # M2_c = Sxx_c - Sx_c^2 / T; groups are 3 consecutive channels.
            gSx = mg_s[:, bs, 8:26]
            gSxx = mg_s[:, bs, 26:44]
            gsq = ft("gsq_g", (P, NBLK, NG))
            V.tensor_mul(gsq[:], gSx, gSx)
            gm2c = ft("gm2c", (P, NBLK, NG))
            V.scalar_tensor_tensor(gm2c[:], gsq[:], -1.0 / T, gSxx,
                               op0=ALU.mult, op1=ALU.add)
            gM2 = ft("gM2", (P, NBLK, 3, 2))
            V.reduce_sum(gM2[:].rearrange("p b g s -> p b (g s)"),
                     gm2c[:].rearrange("p b (j c) -> p b j c", c=3),
                     axis=AX.X)

            vq = out_t[:, bs, 20:32].rearrange("p b (j q) -> p b j q", q=4)

            # ---- Sqrt-set ACT ops ----
            SC.activation(out_t[:, bs, 8:10], m2F[:], AF.Sqrt, scale=1.0 / (T - 1))
            rmsF = ft("rmsF")
            SC.activation(rmsF[:], sqF, AF.Sqrt, scale=1.0 / T)
            rmsS = ft("rmsS")
            SC.activation(rmsS[:], sqS[:], AF.Sqrt, scale=1.0 / T)

            # kurt/skew (uses std at out[...,8:10])
            sg = ft("sg")
            V.tensor_scalar(sg[:], out_t[:, bs, 8:10], 1e-6, None, op0=ALU.max)
            vv = ft("vv")
            V.tensor_mul(vv[:], sg[:], sg[:])
            v4 = ft("v4")
            V.tensor_mul(v4[:], vv[:], vv[:])
            V.reciprocal(v4[:], v4[:])
            kr = ft("kr")
            V.scalar_tensor_tensor(kr[:], m4c[:], 1.0 / T, v4[:],
                               op0=ALU.mult, op1=ALU.mult)
            V.tensor_scalar(out_t[:, bs, 38:40], kr[:], 30.0, -10.0,
                        op0=ALU.min, op1=ALU.max)
            v3 = ft("v3")
            V.tensor_mul(v3[:], vv[:], sg[:])
            V.reciprocal(v3[:], v3[:])
            sk = ft("sk")
            V.scalar_tensor_tensor(sk[:], m3c[:], 1.0 / T, v3[:],
                               op0=ALU.mult, op1=ALU.mult)
            V.tensor_scalar(out_t[:, bs, 40:42], sk[:], 10.0, -10.0,
                        op0=ALU.min, op1=ALU.max)

            # trans arg = rmsS / (rmsF + 1e-6)
            rdn = ft("rdn")
            V.tensor_scalar(rdn[:], rmsF[:], EPS, None, op0=ALU.add)
            V.reciprocal(rdn[:], rdn[:])
            targ = ft("targ")
            V.tensor_mul(targ[:], rmsS[:], rdn[:])

            # ---- Ln-set ACT ops (log1p via bias=1) ----
            SC.activation(out_t[:, bs, 4:6], ratio_arg[:], AF.Ln, bias=1.0)
            SC.activation(out_t[:, bs, 14:16], vra[:], AF.Ln, bias=1.0)
            SC.activation(out_t[:, bs, 36:38], targ[:], AF.Ln, bias=1.0)
            SC.activation(vq[:, :, :, 0:2], gM2[:],
                      AF.Ln, scale=1.0 / (T - 1), bias=1.0)

            # asym_gy = |fg_var - sg_var| (after log1p)
            ag = ft("ag")
            V.tensor_sub(ag[:], out_t[:, bs, 20:22], out_t[:, bs, 24:26])
            SC.activation(out_t[:, bs, 34:36], ag[:], AF.Abs)

            # ---- store ----
            nc.sync.dma_start(
                out=out_d.ap().rearrange("(b p) f -> p b f", p=P)[:, bs],
                in_=out_t[:, bs])


        for b in range(NBLK):
            Xb = p_in.tile([P, NCH, T], F16, tag="xb", name="Xb")
            nc.sync.dma_start(out=Xb[:, 0:4], in_=x_ap[b * P:(b + 1) * P, 0:4])
            G8 = p_g8.tile([P, 2, NG, P], F16, tag="g8", name="G8")
            nc.sync.dma_start(out=G8[:], in_=g8_ap[b])
            XT = p_xt.tile([P, 2, 2, P], F16, tag="xt", name="XT")
            nc.sync.dma_start(out=XT[:], in_=xt_ap[b])
            nc.sync.dma_start(out=Xb[:, 4:NCH],
                              in_=x_ap[b * P:(b + 1) * P, 4:NCH])

            # sz bn_stats first: only needs the tiny x[0:4] DMA.
            # interleaved even/odd: even stats = szL (ch 2), odd = szR (ch 3)
            xa = Xb[:, 2, :]
            xi = _bass.AP(tensor=xa.tensor, offset=xa.offset,
                          ap=[xa.ap[0], [1, T], [T, 2]])
            V.add_instruction(mybir.InstBNStats(
                name=f"I-{nc.next_id()}",
                ins=[V.lower_ap(xi)],
                outs=[V.lower_ap(bnS_s[:, b, :])]))

            # gyro squares for Sxx, split across the three flex engines:
            # ch 0:6 DVE, 6:12 ACT, 12:18 GPSIMD (all both chunks)
            GSQ = p_gsq.tile([P, 2, NG, P], F16, tag="gsq", name="GSQ")
            V.tensor_tensor(GSQ[:, :, 0:6, :], G8[:, :, 0:6, :],
                            G8[:, :, 0:6, :], op=ALU.mult)
            SC.activation(GSQ[:, :, 6:12, :], G8[:, :, 6:12, :], AF.Square)
            G.tensor_tensor(GSQ[:, :, 12:18, :], G8[:, :, 12:18, :],
                            G8[:, :, 12:18, :], op=ALU.mult)

            # ---------------- PE: transposed DFT ----------------
            # F[k, s] per (side, ktile); accumulate over chunks.
            psF = p_psf.tile([P, 2, 2, P], F32, tag="psF", name="psF")
            for side in range(2):
                for j in range(2):
                    for ck in range(2):
                        TE.matmul(psF[:, side, j, :],
                                  w_sb[:, ck, j * P:(j + 1) * P],
                                  XT[:, ck, side, :],
                                  start=(ck == 0), stop=(ck == 1))
            # |F|^2 -> SBUF fp16, one ACT pass
            P2 = p_p2.tile([P, 2, 2, P], F16, tag="p2", name="P2")
            SC.activation(P2[:], psF[:], AF.Square)
            # weighted sums: [s, side, (tot, scn, hf)]
            psW = p_psw.tile([P, 2, 3], F32, tag="psW", name="psW")
            for side in range(2):
                for j in range(2):
                    TE.matmul(psW[:, side, :], P2[:, side, j, :],
                              wv_sb[:, j, :], start=(j == 0), stop=(j == 1))

            # gyro Sx / Sxx ones-matmuls
            psK = p_psk.tile([P, 48], F32, tag="psk", name="psK")
            for c in range(NG):
                for ck in range(2):
                    TE.matmul(psK[:, 8 + c:9 + c], G8[:, ck, c, :], on_sb[:],
                              start=(ck == 0), stop=(ck == 1))
            for c in range(NG):
                for ck in range(2):
                    TE.matmul(psK[:, 26 + c:27 + c], GSQ[:, ck, c, :],
                              on_sb[:], start=(ck == 0), stop=(ck == 1))

            # ---------------- DVE: peaks (abs + max tree) ----------------
            ABS = p_abs.tile([P, NCH, T], F16, tag="absx", name="ABS")
            V.tensor_scalar(ABS[:].bitcast(I16), Xb[:].bitcast(I16),
                            0x7FFF, None, op0=ALU.bitwise_and)
            L1 = p_tree.tile([P, NCH, 128], F16, tag="l1", name="L1")
            V.tensor_tensor(L1[:], ABS[:, :, 0:128], ABS[:, :, 128:256],
                            op=ALU.max)
            L2 = p_tree.tile([P, NCH, 64], F16, tag="l2", name="L2")
            V.tensor_tensor(L2[:], L1[:, :, 0:64], L1[:, :, 64:128],
                            op=ALU.max)
            L3 = p_tree.tile([P, NCH, 32], F16, tag="l3", name="L3")
            V.tensor_tensor(L3[:], L2[:, :, 0:32], L2[:, :, 32:64],
                            op=ALU.max)
            L4 = p_tree.tile([P, NCH, 16], F16, tag="l4", name="L4")
            V.tensor_tensor(L4[:], L3[:, :, 0:16], L3[:, :, 16:32],
                            op=ALU.max)
            # fz/sz peaks -> out cols 0:4 (f_pk L/R, s_pk L/R)
            V.tensor_reduce(out_t[:, b, 0:4], L4[:, 0:4, :], axis=AX.X,
                            op=ALU.max)
            # gyro group peaks -> out cols 22:24, 26:28, 30:32
            vq = out_t[:, b, 20:32].rearrange("p (j q) -> p j q", q=4)
            V.tensor_reduce(vq[:, :, 2:4],
                            L4[:, 4:22, :].rearrange(
                                "p (g c) t -> p g c t", g=6),
                            axis=AX.XY, op=ALU.max)

            # |XT| for half-wave sums, powers of fz (t-layout)
            AXT = p_pow.tile([P, 2, 2, P], F16, tag="axt", name="AXT")
            V.tensor_scalar(AXT[:].bitcast(I16), XT[:].bitcast(I16),
                            0x7FFF, None, op0=ALU.bitwise_and)
            XT2 = p_pow.tile([P, 2, 2, P], F16, tag="xt2", name="XT2")
            SC.activation(XT2[:], XT[:], AF.Square)
            XT3 = p_pow.tile([P, 2, 2, P], F16, tag="xt3", name="XT3")
            V.tensor_tensor(XT3[:], XT2[:], XT[:], op=ALU.mult)
            XT4 = p_pow.tile([P, 2, 2, P], F16, tag="xt4", name="XT4")
            SC.activation(XT4[:], XT2[:], AF.Square)
            for side in range(2):
                for ck in range(2):
                    TE.matmul(psK[:, 2 * ck + side:2 * ck + side + 1],
                              AXT[:, ck, side, :], on_sb[:],
                              start=True, stop=True)
            for side in range(2):
                for ck in range(2):
                    TE.matmul(psK[:, 4 + side:5 + side],
                              XT3[:, ck, side, :], on_sb[:],
                              start=(ck == 0), stop=(ck == 1))
            for side in range(2):
                for ck in range(2):
                    TE.matmul(psK[:, 6 + side:7 + side],
                              XT4[:, ck, side, :], on_sb[:],
                              start=(ck == 0), stop=(ck == 1))
            # SxF, sqF (fz sum and sum of squares)
            for side in range(2):
                for ck in range(2):
                    TE.matmul(psK[:, 44 + side:45 + side],
                              XT[:, ck, side, :], on_sb[:],
                              start=(ck == 0), stop=(ck == 1))
            for side in range(2):
                for ck in range(2):
                    TE.matmul(psK[:, 46 + side:47 + side],
                              XT2[:, ck, side, :], on_sb[:],
                              start=(ck == 0), stop=(ck == 1))

            # ---------------- fz scalar features ----------------
            i2 = 2 * b

            # impact: sign(|fz| - 0.3 pk), thr per partition
            thr = p_small.tile([P, 2], F32, tag="thr", name="thr")
            V.tensor_scalar(thr[:], out_t[:, b, 0:2], -0.3, None,
                            op0=ALU.mult)
            for side in range(2):
                j1 = p_junk.tile([P, T], F16, tag="j1", name="j1")
                SC.activation(j1[:], ABS[:, side, :], AF.Sign,
                              bias=thr[:, side:side + 1],
                              accum_out=imp_s[:, i2 + side:i2 + side + 1])

            # zcr: sign(x_t * x_{t+1})
            PR = p_scr.tile([P, 2, T - 1], F16, tag="pr", name="PR")
            V.tensor_tensor(PR[:], Xb[:, 0:2, 1:T], Xb[:, 0:2, 0:T - 1],
                            op=ALU.mult)
            for side in range(2):
                j2 = p_junk.tile([P, T], F16, tag="j2", name="j2")
                SC.activation(j2[:, 0:T - 1], PR[:, side, :], AF.Sign,
                              accum_out=zc_s[:, i2 + side:i2 + side + 1])

            # vib: |sz_t - sz_{t-1}|
            D = p_scr.tile([P, 2, T - 1], F16, tag="d", name="D")
            V.tensor_tensor(D[:], Xb[:, 2:4, 1:T], Xb[:, 2:4, 0:T - 1],
                            op=ALU.subtract)
            for side in range(2):
                j3 = p_junk.tile([P, T], F16, tag="j3", name="j3")
                SC.activation(j3[:, 0:T - 1], D[:, side, :], AF.Abs,
                              accum_out=vib_s[:, i2 + side:i2 + side + 1])

            # stats out of PSUM
            V.tensor_copy(mg_s[:, b, :], psK[:])
            SC.activation(thw_s[:, b], psW[:], AF.Copy)

        emit_final(0, NBLK)


_NC_CACHE = None
_CONSTS = None


def _get_nc():
    global _NC_CACHE, _CONSTS
    if _NC_CACHE is None:
        _NC_CACHE = build_nc()
    if _CONSTS is None:
        _CONSTS = build_consts()
    return _NC_CACHE, _CONSTS


def run(foot, shank, thigh, **kw):
    arrs = {"foot": np.asarray(foot), "shank": np.asarray(shank),
            "thigh": np.asarray(thigh)}
    X = np.empty((B_FULL, NCH, T), dtype=np.float16)
    for j, (name, ch) in enumerate(SRC):
        X[:, j] = arrs[name][:, ch]

    nc, consts = _get_nc()
    in_maps = []
    for i in range(N_CORES):
        Xc = X[i * BC:(i + 1) * BC]
        fz = Xc[:, 0:2, :].reshape(NBLK, P, 2, 2, P)   # [b, s, side, ck, t]
        xt = np.ascontiguousarray(fz.transpose(0, 4, 3, 2, 1))
        gy = Xc[:, 4:22, :].reshape(NBLK, P, NG, 2, P)  # [b, s, c, ck, t]
        g8 = np.ascontiguousarray(gy.transpose(0, 4, 3, 2, 1))
        in_maps.append({
            "x": np.ascontiguousarray(Xc),
            "xt": xt,
            "g8": g8,
            "w": consts["w"], "wv": consts["wv"], "ones": consts["ones"],
        })
    return run_bass_kernel_spmd(nc, in_maps, core_ids=list(range(N_CORES)),
                                **kw)


def kernel(foot, shank, thigh):
    res = run(foot, shank, thigh)
    return np.concatenate([res.results[i]["out"] for i in range(N_CORES)],
                          axis=0)


# revision 28
# speedup vs baseline: 1.1477x; 1.1477x over previous
"""BioMech feature extraction on Trainium2: 8 NeuronCores, pure data-parallel.

Self-contained: takes full inputs foot/shank/thigh [8192, 12, 256] fp32,
returns [8192, 44] fp32 feature matrix matching the reference stack order.

V1 design (vs. earlier baseline):
  - Transposed DFT: stationary = DFT weight k-tiles [t, k], moving = fz_T
    [t, s] -> F[k, s] in PSUM.  |F|^2 via one ACT Square (PSUM->SBUF fp16),
    then tot/scn/hf all come from a single pair of weighted-sum matmuls
    (moving = [ones | k | hf-mask]) landing [s, 3] sample-major in PSUM.
    Kills the per-side ACT square+accum calls, the hf DVE reduce, the
    sqrt(k) DFT columns, and the muN extraction.
  - Peak tree: first level is tensor_tensor(abs_max) straight off the raw
    fp16 block, killing the full-width |x| mask pass.  Small |fz| mask kept
    for the impact feature only.
  - Sum x and sum x^2 for fz via ones-matmuls over XT / XT2 (exact, replaces
    the Parseval reconstruction).
  - GSQ (gyro squares for Sxx) split three ways: 1/3 DVE tensor_tensor,
    1/3 ACT Square, 1/3 GPSIMD tensor_tensor (the idle engine).
  - Everything else (impact/zcr/vib ACT sign/abs accums, bn_stats shank
    pair, gyro Sx/Sxx ones-matmuls, powers for moments) as in baseline.
"""

import contextlib

import numpy as np

import concourse.bacc as bacc
import concourse.bass as _bass
import concourse.tile as tile
import concourse.mybir as mybir
from concourse.bass_utils import run_bass_kernel_spmd

F32 = mybir.dt.float32
F16 = mybir.dt.float16
I16 = mybir.dt.int16
AF = mybir.ActivationFunctionType
ALU = mybir.AluOpType
AX = mybir.AxisListType

N_CORES = 8
B_FULL = 8192
T = 256
P = 128
BC = B_FULL // N_CORES          # 1024 samples per core
NBLK = BC // P                  # 8 blocks
NCH = 22
NBINS = 129
HF_BIN = 60
NK = 256                        # transposed-DFT k columns (2 tiles of 128)
EPS = 1e-6
NS = NBLK * 2                   # (block, side) stat slots

# packed channel order:
# 0 fzL, 1 fzR, 2 szL, 3 szR, 4:7 fgL, 7:10 fgR, 10:13 sgL, 13:16 sgR,
# 16:19 tgL, 19:22 tgR
SRC = [("foot", 2), ("foot", 8), ("shank", 2), ("shank", 8),
       ("foot", 3), ("foot", 4), ("foot", 5), ("foot", 9), ("foot", 10), ("foot", 11),
       ("shank", 3), ("shank", 4), ("shank", 5), ("shank", 9), ("shank", 10), ("shank", 11),
       ("thigh", 3), ("thigh", 4), ("thigh", 5), ("thigh", 9), ("thigh", 10), ("thigh", 11)]

NG = 18                         # gyro channels (packed 4..21)


def build_consts():
    t = np.arange(T, dtype=np.float64)
    # transposed-DFT weights: W[ck, trow, j] = basis_j(ck*128 + trow)
    # cols: tile0 = C_0..C_127; tile1 = [C_128, S_1..S_127]
    k_c = np.arange(128, dtype=np.float64)              # tile0 cos ks
    ang_c = 2.0 * np.pi * np.outer(t, k_c) / T
    tile0 = np.cos(ang_c)                               # [256, 128]
    k_s = np.arange(1, 128, dtype=np.float64)
    ang_s = 2.0 * np.pi * np.outer(t, k_s) / T
    c128 = np.cos(np.pi * t)[:, None]                   # (-1)^t
    tile1 = np.concatenate([c128, np.sin(ang_s)], axis=1)   # [256, 128]
    w = np.concatenate([tile0, tile1], axis=1)          # [256, 256]
    w = np.ascontiguousarray(w.reshape(2, P, NK), dtype=np.float16)

    # weighted-sum moving vectors per k-tile: [ones | k | hf-mask]
    wv = np.zeros((2, P, 3), dtype=np.float16)
    wv[0, :, 0] = 1.0
    wv[0, :, 1] = np.arange(128)
    wv[0, :, 2] = (np.arange(128) >= HF_BIN)
    wv[1, :, 0] = 1.0
    wv[1, 0, 1] = 128.0
    wv[1, 1:, 1] = np.arange(1, 128)
    wv[1, 0, 2] = 1.0
    wv[1, 1:, 2] = (np.arange(1, 128) >= HF_BIN)

    ones = np.ones((P, 1), dtype=np.float16)
    return {"w": w, "wv": wv, "ones": ones}


def build_nc():
    nc = bacc.Bacc("TRN2", target_bir_lowering=False, debug=False,
                   num_devices=N_CORES)
    x_d = nc.dram_tensor("x", [BC, NCH, T], F16, kind="ExternalInput")
    xt_d = nc.dram_tensor("xt", [NBLK, P, 2, 2, P], F16, kind="ExternalInput")
    g8_d = nc.dram_tensor("g8", [NBLK, P, 2, NG, P], F16,
                          kind="ExternalInput")
    w_d = nc.dram_tensor("w", [2, P, NK], F16, kind="ExternalInput")
    wv_d = nc.dram_tensor("wv", [2, P, 3], F16, kind="ExternalInput")
    on_d = nc.dram_tensor("ones", [P, 1], F16, kind="ExternalInput")
    out_d = nc.dram_tensor("out", [BC, 44], F32, kind="ExternalOutput")

    with tile.TileContext(nc) as tc:
        _body(tc, x_d, xt_d, g8_d, w_d, wv_d, on_d, out_d)
    nc.compile()
    return nc


def _body(tc, x_d, xt_d, g8_d, w_d, wv_d, on_d, out_d):
    nc = tc.nc
    ctx = contextlib.ExitStack()
    with ctx:
        pers = ctx.enter_context(tc.tile_pool(name="pers", bufs=1))
        p_in = ctx.enter_context(tc.tile_pool(name="xin", bufs=3))
        p_xt = ctx.enter_context(tc.tile_pool(name="xtin", bufs=3))
        p_g8 = ctx.enter_context(tc.tile_pool(name="g8in", bufs=3))
        p_gsq = ctx.enter_context(tc.tile_pool(name="gsq", bufs=2))
        p_abs = ctx.enter_context(tc.tile_pool(name="abs", bufs=2))
        p_tree = ctx.enter_context(tc.tile_pool(name="tree", bufs=2))
        p_pow = ctx.enter_context(tc.tile_pool(name="pow", bufs=2))
        p_p2 = ctx.enter_context(tc.tile_pool(name="p2", bufs=2))
        p_scr = ctx.enter_context(tc.tile_pool(name="scr", bufs=2))
        p_junk = ctx.enter_context(tc.tile_pool(name="junk", bufs=4))
        p_small = ctx.enter_context(tc.tile_pool(name="small", bufs=3))
        p_psf = ctx.enter_context(tc.tile_pool(name="psf", bufs=2, space="PSUM"))
        p_psw = ctx.enter_context(tc.tile_pool(name="psw", bufs=2, space="PSUM"))
        p_psk = ctx.enter_context(tc.tile_pool(name="psk", bufs=2, space="PSUM"))
        fin = ctx.enter_context(tc.tile_pool(name="fin", bufs=1))

        V = nc.vector
        SC = nc.scalar
        G = nc.gpsimd
        TE = nc.tensor

        # ---- constants ----
        w_sb = pers.tile([P, 2, NK], F16, tag="w", name="w_sb")
        wv_sb = pers.tile([P, 2, 3], F16, tag="wv", name="wv_sb")
        on_sb = pers.tile([P, 1], F16, tag="ones", name="on_sb")
        nc.sync.dma_start(out=w_sb[:], in_=w_d.ap().rearrange("j p n -> p j n"))
        nc.sync.dma_start(out=wv_sb[:], in_=wv_d.ap().rearrange("j p n -> p j n"))
        nc.sync.dma_start(out=on_sb[:], in_=on_d.ap())

        def stat(tag, n=NS):
            return pers.tile([P, n], F32, tag=tag, name=tag)

        # thw[:, b, side, 0:3] = [tot, scn, hf]
        thw_s = pers.tile([P, NBLK, 2, 3], F32, tag="thw", name="thw")
        imp_s = stat("imp")     # sum sign(|fz| - 0.3pk)
        zc_s = stat("zc")       # sum sign(x_t * x_{t+1})
        vib_s = stat("vib")     # sum |diff sz|
        # psK cols: 0,1 habs0 (L,R); 2,3 habs1 (L,R); 4,5 Sx3; 6,7 Sx4;
        # 8:26 gyro Sx; 26:44 gyro Sxx; 44,45 SxF (L,R); 46,47 sqF (L,R)
        mg_s = pers.tile([P, NBLK, 48], F32, tag="mg", name="mg_s")
        bnS_s = pers.tile([P, NBLK, 6], F32, tag="bnS", name="bnS")
        out_t = pers.tile([P, NBLK, 44], F32, tag="out", name="out_t")

        # preload the Sqrt/Ln ACT table sets during the DMA-fill startup so
        # the final phase doesn't pay the table-load latency.
        warm = pers.tile([P, 2], F32, tag="warm", name="warm")
        nc.vector.memset(warm[:], 1.0)
        SC.activation(warm[:, 0:1], warm[:, 1:2], AF.Sqrt)
        SC.activation(warm[:, 0:1], warm[:, 1:2], AF.Ln)

        x_ap = x_d.ap()
        xt_ap = xt_d.ap()
        g8_ap = g8_d.ap()

        # ============ final batched scalar phase (block range) ============
        def emit_final(b0, b1):
            bs = slice(b0, b1)

            def v2(tbl):
                return tbl[:].rearrange("p (b s) -> p b s", s=2)[:, bs, :]

            def ft(tag, shape=(P, NBLK, 2)):
                t = fin.tile(list(shape), F32, tag=tag, name=tag)
                return t[:, bs]

            # ratio = log1p(f_pk / (s_pk + 1e-4))  [Ln later]
            r_spk = ft("r_spk")
            V.tensor_scalar(r_spk[:], out_t[:, bs, 2:4], 1e-4, None, op0=ALU.add)
            V.reciprocal(r_spk[:], r_spk[:])
            ratio_arg = ft("ratio_arg")
            V.tensor_mul(ratio_arg[:], out_t[:, bs, 0:2], r_spk[:])

            # total power reciprocal (shared by hf and sc)
            r_tot = ft("r_tot")
            V.tensor_scalar(r_tot[:], thw_s[:, bs, :, 0], EPS, None, op0=ALU.add)
            V.reciprocal(r_tot[:], r_tot[:])
            V.tensor_mul(out_t[:, bs, 6:8], thw_s[:, bs, :, 2], r_tot[:])
            V.scalar_tensor_tensor(out_t[:, bs, 16:18], thw_s[:, bs, :, 1],
                               1.0 / NBINS, r_tot[:],
                               op0=ALU.mult, op1=ALU.mult)

            # dur, vib, zcr scaled counts
            V.tensor_scalar(out_t[:, bs, 18:20], v2(imp_s), 1.0 / (2 * T), 0.5,
                        op0=ALU.mult, op1=ALU.add)
            V.tensor_scalar(out_t[:, bs, 12:14], v2(vib_s), 1.0 / (T - 1), None,
                        op0=ALU.mult)
            V.tensor_scalar(out_t[:, bs, 42:44], v2(zc_s), -1.0 / (2 * (T - 1)),
                        0.5, op0=ALU.mult, op1=ALU.add)

            # decay = h0 / (h1 + 128e-6)
            hv = mg_s[:, bs, 0:4].rearrange("p b (h s) -> p b h s", h=2)
            dden = ft("dden")
            V.tensor_scalar(dden[:], hv[:, :, 1, :], (T // 2) * EPS, None,
                        op0=ALU.add)
            V.reciprocal(dden[:], dden[:])
            V.tensor_mul(out_t[:, bs, 10:12], hv[:, :, 0, :], dden[:])

            # asym_acc = |f_pk - s_pk|
            aa = ft("aa")
            V.tensor_sub(aa[:], out_t[:, bs, 0:2], out_t[:, bs, 2:4])
            SC.activation(out_t[:, bs, 32:34], aa[:], AF.Abs)

            # fz sums: mu = SxF/T; sqF = sum x^2; m2F = sqF - SxF^2/T
            SxF = mg_s[:, bs, 44:46]
            sqF = mg_s[:, bs, 46:48]
            mu = ft("mu")
            V.tensor_scalar(mu[:], SxF, 1.0 / T, None, op0=ALU.mult)
            qF = ft("qF")                 # mu^2
            V.tensor_mul(qF[:], mu[:], mu[:])
            m2F = ft("m2F")               # sum (x-mu)^2
            V.scalar_tensor_tensor(m2F[:], qF[:], -float(T), sqF,
                               op0=ALU.mult, op1=ALU.add)

            # central moments from raw sums:
            # m3c = Sx3 - 3 mu sqF + 2 T mu^3
            # m4c = Sx4 - 4 mu Sx3 + 6 mu^2 sqF - 3 T mu^4
            Sx3 = mg_s[:, bs, 4:6]
            Sx4 = mg_s[:, bs, 6:8]
            mu2 = qF
            muSq = ft("muSq")                       # mu * sqF
            V.tensor_mul(muSq[:], mu[:], sqF)
            mu3 = ft("mu3")
            V.tensor_mul(mu3[:], mu2[:], mu[:])
            t1 = ft("t1")                           # Sx3 - 3 mu sqF
            V.scalar_tensor_tensor(t1[:], muSq[:], -3.0, Sx3,
                               op0=ALU.mult, op1=ALU.add)
            m3c = ft("m3c")
            V.scalar_tensor_tensor(m3c[:], mu3[:], 2.0 * T, t1[:],
                               op0=ALU.mult, op1=ALU.add)
            muSx3 = ft("muSx3")
            V.tensor_mul(muSx3[:], mu[:], Sx3)
            mu2sq = ft("mu2sq")                     # mu^2 * sqF
            V.tensor_mul(mu2sq[:], mu2[:], sqF)
            mu4 = ft("mu4")
            V.tensor_mul(mu4[:], mu2[:], mu2[:])
            t2 = ft("t2")                           # Sx4 - 4 mu Sx3
            V.scalar_tensor_tensor(t2[:], muSx3[:], -4.0, Sx4,
                               op0=ALU.mult, op1=ALU.add)
            t3 = ft("t3")                           # + 6 mu^2 sqF
            V.scalar_tensor_tensor(t3[:], mu2sq[:], 6.0, t2[:],
                               op0=ALU.mult, op1=ALU.add)
            m4c = ft("m4c")
            V.scalar_tensor_tensor(m4c[:], mu4[:], -3.0 * T, t3[:],
                               op0=ALU.mult, op1=ALU.add)

            # shank stats from bn call 0: even = szL, odd = szR
            meS = bnS_s[:, bs, 1:5:3]               # [p, b, 2] means (cols 1,4)
            M2S = bnS_s[:, bs, 2:6:3]               # [p, b, 2] sum (y-mu)^2 (cols 2,5)
            sqS = ft("sqS")                         # sum y^2 = M2 + T mu^2
            uS = ft("uS")
            V.tensor_mul(uS[:], meS, meS)
            V.scalar_tensor_tensor(sqS[:], uS[:], float(T), M2S,
                               op0=ALU.mult, op1=ALU.add)

            # var_ratio = log1p(m2F / (M2S + 255e-4))  [Ln later]
            vr = ft("vr")
            V.tensor_scalar(vr[:], M2S, (T - 1) * 1e-4, None, op0=ALU.add)
            V.reciprocal(vr[:], vr[:])
            vra = ft("vra")
            V.tensor_mul(vra[:], m2F[:], vr[:])

            # gyro M2 group sums from the PE Sx/Sxx stats:
            # M2_c = Sxx_c - Sx_c^2 / T; groups are 3 consecutive channels.
            gSx = mg_s[:, bs, 8:26]
            gSxx = mg_s[:, bs, 26:44]
            gsq = ft("gsq_g", (P, NBLK, NG))
            V.tensor_mul(gsq[:], gSx, gSx)
            gm2c = ft("gm2c", (P, NBLK, NG))
            V.scalar_tensor_tensor(gm2c[:], gsq[:], -1.0 / T, gSxx,
                               op0=ALU.mult, op1=ALU.add)
            gM2 = ft("gM2", (P, NBLK, 3, 2))
            V.reduce_sum(gM2[:].rearrange("p b g s -> p b (g s)"),
                     gm2c[:].rearrange("p b (j c) -> p b j c", c=3),
                     axis=AX.X)

            vq = out_t[:, bs, 20:32].rearrange("p b (j q) -> p b j q", q=4)

            # ---- Sqrt-set ACT ops ----
            SC.activation(out_t[:, bs, 8:10], m2F[:], AF.Sqrt, scale=1.0 / (T - 1))
            rmsF = ft("rmsF")
            SC.activation(rmsF[:], sqF, AF.Sqrt, scale=1.0 / T)
            rmsS = ft("rmsS")
            SC.activation(rmsS[:], sqS[:], AF.Sqrt, scale=1.0 / T)

            # kurt/skew (uses std at out[...,8:10])
            sg = ft("sg")
            V.tensor_scalar(sg[:], out_t[:, bs, 8:10], 1e-6, None, op0=ALU.max)
            vv = ft("vv")
            V.tensor_mul(vv[:], sg[:], sg[:])
            v4 = ft("v4")
            V.tensor_mul(v4[:], vv[:], vv[:])
            V.reciprocal(v4[:], v4[:])
            kr = ft("kr")
            V.scalar_tensor_tensor(kr[:], m4c[:], 1.0 / T, v4[:],
                               op0=ALU.mult, op1=ALU.mult)
            V.tensor_scalar(out_t[:, bs, 38:40], kr[:], 30.0, -10.0,
                        op0=ALU.min, op1=ALU.max)
            v3 = ft("v3")
            V.tensor_mul(v3[:], vv[:], sg[:])
            V.reciprocal(v3[:], v3[:])
            sk = ft("sk")
            V.scalar_tensor_tensor(sk[:], m3c[:], 1.0 / T, v3[:],
                               op0=ALU.mult, op1=ALU.mult)
            V.tensor_scalar(out_t[:, bs, 40:42], sk[:], 10.0, -10.0,
                        op0=ALU.min, op1=ALU.max)

            # trans arg = rmsS / (rmsF + 1e-6)
            rdn = ft("rdn")
            V.tensor_scalar(rdn[:], rmsF[:], EPS, None, op0=ALU.add)
            V.reciprocal(rdn[:], rdn[:])
            targ = ft("targ")
            V.tensor_mul(targ[:], rmsS[:], rdn[:])

            # ---- Ln-set ACT ops (log1p via bias=1) ----
            SC.activation(out_t[:, bs, 4:6], ratio_arg[:], AF.Ln, bias=1.0)
            SC.activation(out_t[:, bs, 14:16], vra[:], AF.Ln, bias=1.0)
            SC.activation(out_t[:, bs, 36:38], targ[:], AF.Ln, bias=1.0)
            SC.activation(vq[:, :, :, 0:2], gM2[:],
                      AF.Ln, scale=1.0 / (T - 1), bias=1.0)

            # asym_gy = |fg_var - sg_var| (after log1p)
            ag = ft("ag")
            V.tensor_sub(ag[:], out_t[:, bs, 20:22], out_t[:, bs, 24:26])
            SC.activation(out_t[:, bs, 34:36], ag[:], AF.Abs)

            # ---- store ----
            nc.sync.dma_start(
                out=out_d.ap().rearrange("(b p) f -> p b f", p=P)[:, bs],
                in_=out_t[:, bs])


        for b in range(NBLK):
            # fz/sz channels in their own tile so bn/tree/impact can start
            # as soon as the small 256 KB transfer lands
            X4 = p_in.tile([P, 4, T], F16, tag="x4", name="X4")
            nc.sync.dma_start(out=X4[:], in_=x_ap[b * P:(b + 1) * P, 0:4])
            XT = p_xt.tile([P, 2, 2, P], F16, tag="xt", name="XT")
            nc.sync.dma_start(out=XT[:], in_=xt_ap[b])
            G8 = p_g8.tile([P, 2, NG, P], F16, tag="g8", name="G8")
            nc.sync.dma_start(out=G8[:], in_=g8_ap[b])
            XG = p_in.tile([P, NG, T], F16, tag="xg", name="XG")
            nc.sync.dma_start(out=XG[:], in_=x_ap[b * P:(b + 1) * P, 4:NCH])

            # sz bn_stats first: only needs the tiny x4 DMA.
            # interleaved even/odd: even stats = szL (ch 2), odd = szR (ch 3)
            xa = X4[:, 2, :]
            xi = _bass.AP(tensor=xa.tensor, offset=xa.offset,
                          ap=[xa.ap[0], [1, T], [T, 2]])
            V.add_instruction(mybir.InstBNStats(
                name=f"I-{nc.next_id()}",
                ins=[V.lower_ap(xi)],
                outs=[V.lower_ap(bnS_s[:, b, :])]))

            # gyro squares for Sxx, split DVE/ACT (GPSIMD offload measurably
            # slows the DVE via SBUF-port contention — keep GPSIMD idle)
            GSQ = p_gsq.tile([P, 2, NG, P], F16, tag="gsq", name="GSQ")
            V.tensor_tensor(GSQ[:, :, 0:6, :], G8[:, :, 0:6, :],
                            G8[:, :, 0:6, :], op=ALU.mult)
            SC.activation(GSQ[:, :, 6:NG, :], G8[:, :, 6:NG, :], AF.Square)

            # ---------------- PE: transposed DFT ----------------
            # F[k, s] per (side, ktile); accumulate over chunks.
            psF = p_psf.tile([P, 2, 2, P], F32, tag="psF", name="psF")
            for side in range(2):
                for j in range(2):
                    for ck in range(2):
                        TE.matmul(psF[:, side, j, :],
                                  w_sb[:, ck, j * P:(j + 1) * P],
                                  XT[:, ck, side, :],
                                  start=(ck == 0), stop=(ck == 1))
            # |F|^2 -> SBUF fp16, one ACT pass
            P2 = p_p2.tile([P, 2, 2, P], F16, tag="p2", name="P2")
            SC.activation(P2[:], psF[:], AF.Square)
            # weighted sums: [s, side, (tot, scn, hf)]
            psW = p_psw.tile([P, 2, 3], F32, tag="psW", name="psW")
            for side in range(2):
                for j in range(2):
                    TE.matmul(psW[:, side, :], P2[:, side, j, :],
                              wv_sb[:, j, :], start=(j == 0), stop=(j == 1))

            # gyro Sx / Sxx ones-matmuls
            psK = p_psk.tile([P, 48], F32, tag="psk", name="psK")
            for c in range(NG):
                for ck in range(2):
                    TE.matmul(psK[:, 8 + c:9 + c], G8[:, ck, c, :], on_sb[:],
                              start=(ck == 0), stop=(ck == 1))
            for c in range(NG):
                for ck in range(2):
                    TE.matmul(psK[:, 26 + c:27 + c], GSQ[:, ck, c, :],
                              on_sb[:], start=(ck == 0), stop=(ck == 1))

            # ---------------- DVE: peaks (abs + max tree) ----------------
            ABS = p_abs.tile([P, NCH, T], F16, tag="absx", name="ABS")
            V.tensor_scalar(ABS[:, 0:4].bitcast(I16), X4[:].bitcast(I16),
                            0x7FFF, None, op0=ALU.bitwise_and)
            V.tensor_scalar(ABS[:, 4:NCH].bitcast(I16), XG[:].bitcast(I16),
                            0x7FFF, None, op0=ALU.bitwise_and)
            L1 = p_tree.tile([P, NCH, 128], F16, tag="l1", name="L1")
            V.tensor_tensor(L1[:], ABS[:, :, 0:128], ABS[:, :, 128:256],
                            op=ALU.max)
            L2 = p_tree.tile([P, NCH, 64], F16, tag="l2", name="L2")
            V.tensor_tensor(L2[:], L1[:, :, 0:64], L1[:, :, 64:128],
                            op=ALU.max)
            L3 = p_tree.tile([P, NCH, 32], F16, tag="l3", name="L3")
            V.tensor_tensor(L3[:], L2[:, :, 0:32], L2[:, :, 32:64],
                            op=ALU.max)
            L4 = p_tree.tile([P, NCH, 16], F16, tag="l4", name="L4")
            V.tensor_tensor(L4[:], L3[:, :, 0:16], L3[:, :, 16:32],
                            op=ALU.max)
            # fz/sz peaks -> out cols 0:4 (f_pk L/R, s_pk L/R)
            V.tensor_reduce(out_t[:, b, 0:4], L4[:, 0:4, :], axis=AX.X,
                            op=ALU.max)
            # gyro group peaks -> out cols 22:24, 26:28, 30:32
            vq = out_t[:, b, 20:32].rearrange("p (j q) -> p j q", q=4)
            V.tensor_reduce(vq[:, :, 2:4],
                            L4[:, 4:22, :].rearrange(
                                "p (g c) t -> p g c t", g=6),
                            axis=AX.XY, op=ALU.max)

            # |XT| for half-wave sums, powers of fz (t-layout)
            AXT = p_pow.tile([P, 2, 2, P], F16, tag="axt", name="AXT")
            V.tensor_scalar(AXT[:].bitcast(I16), XT[:].bitcast(I16),
                            0x7FFF, None, op0=ALU.bitwise_and)
            XT2 = p_pow.tile([P, 2, 2, P], F16, tag="xt2", name="XT2")
            SC.activation(XT2[:], XT[:], AF.Square)
            XT3 = p_pow.tile([P, 2, 2, P], F16, tag="xt3", name="XT3")
            V.tensor_tensor(XT3[:], XT2[:], XT[:], op=ALU.mult)
            XT4 = p_pow.tile([P, 2, 2, P], F16, tag="xt4", name="XT4")
            SC.activation(XT4[:], XT2[:], AF.Square)
            for side in range(2):
                for ck in range(2):
                    TE.matmul(psK[:, 2 * ck + side:2 * ck + side + 1],
                              AXT[:, ck, side, :], on_sb[:],
                              start=True, stop=True)
            for side in range(2):
                for ck in range(2):
                    TE.matmul(psK[:, 4 + side:5 + side],
                              XT3[:, ck, side, :], on_sb[:],
                              start=(ck == 0), stop=(ck == 1))
            for side in range(2):
                for ck in range(2):
                    TE.matmul(psK[:, 6 + side:7 + side],
                              XT4[:, ck, side, :], on_sb[:],
                              start=(ck == 0), stop=(ck == 1))
            # SxF, sqF (fz sum and sum of squares)
            for side in range(2):
                for ck in range(2):
                    TE.matmul(psK[:, 44 + side:45 + side],
                              XT[:, ck, side, :], on_sb[:],
                              start=(ck == 0), stop=(ck == 1))
            for side in range(2):
                for ck in range(2):
                    TE.matmul(psK[:, 46 + side:47 + side],
                              XT2[:, ck, side, :], on_sb[:],
                              start=(ck == 0), stop=(ck == 1))

            # ---------------- fz scalar features ----------------
            i2 = 2 * b

            # impact: sign(|fz| - 0.3 pk), thr per partition
            thr = p_small.tile([P, 2], F32, tag="thr", name="thr")
            V.tensor_scalar(thr[:], out_t[:, b, 0:2], -0.3, None,
                            op0=ALU.mult)
            for side in range(2):
                j1 = p_junk.tile([P, T], F16, tag="j1", name="j1")
                SC.activation(j1[:], ABS[:, side, :], AF.Sign,
                              bias=thr[:, side:side + 1],
                              accum_out=imp_s[:, i2 + side:i2 + side + 1])

            # zcr: sign(x_t * x_{t+1})
            PR = p_scr.tile([P, 2, T - 1], F16, tag="pr", name="PR")
            V.tensor_tensor(PR[:], X4[:, 0:2, 1:T], X4[:, 0:2, 0:T - 1],
                            op=ALU.mult)
            for side in range(2):
                j2 = p_junk.tile([P, T], F16, tag="j2", name="j2")
                SC.activation(j2[:, 0:T - 1], PR[:, side, :], AF.Sign,
                              accum_out=zc_s[:, i2 + side:i2 + side + 1])

            # vib: |sz_t - sz_{t-1}|
            D = p_scr.tile([P, 2, T - 1], F16, tag="d", name="D")
            V.tensor_tensor(D[:], X4[:, 2:4, 1:T], X4[:, 2:4, 0:T - 1],
                            op=ALU.subtract)
            for side in range(2):
                j3 = p_junk.tile([P, T], F16, tag="j3", name="j3")
                SC.activation(j3[:, 0:T - 1], D[:, side, :], AF.Abs,
                              accum_out=vib_s[:, i2 + side:i2 + side + 1])

            # stats out of PSUM (ACT copies; DVE is the bottleneck)
            SC.activation(mg_s[:, b, :], psK[:], AF.Copy)
            SC.activation(thw_s[:, b], psW[:], AF.Copy)

            # overlap the first half of the final phase with block 4-7 compute
            if b == NBLK // 2 - 1:
                emit_final(0, NBLK // 2)

        emit_final(NBLK // 2, NBLK)


_NC_CACHE = None
_CONSTS = None


def _get_nc():
    global _NC_CACHE, _CONSTS
    if _NC_CACHE is None:
        _NC_CACHE = build_nc()
    if _CONSTS is None:
        _CONSTS = build_consts()
    return _NC_CACHE, _CONSTS


def run(foot, shank, thigh, **kw):
    arrs = {"foot": np.asarray(foot), "shank": np.asarray(shank),
            "thigh": np.asarray(thigh)}
    X = np.empty((B_FULL, NCH, T), dtype=np.float16)
    for j, (name, ch) in enumerate(SRC):
        X[:, j] = arrs[name][:, ch]

    nc, consts = _get_nc()
    in_maps = []
    for i in range(N_CORES):
        Xc = X[i * BC:(i + 1) * BC]
        fz = Xc[:, 0:2, :].reshape(NBLK, P, 2, 2, P)   # [b, s, side, ck, t]
        xt = np.ascontiguousarray(fz.transpose(0, 4, 3, 2, 1))
        gy = Xc[:, 4:22, :].reshape(NBLK, P, NG, 2, P)  # [b, s, c, ck, t]
        g8 = np.ascontiguousarray(gy.transpose(0, 4, 3, 2, 1))
        in_maps.append({
            "x": np.ascontiguousarray(Xc),
            "xt": xt,
            "g8": g8,
            "w": consts["w"], "wv": consts["wv"], "ones": consts["ones"],
        })
    return run_bass_kernel_spmd(nc, in_maps, core_ids=list(range(N_CORES)),
                                **kw)


def kernel(foot, shank, thigh):
    res = run(foot, shank, thigh)
    return np.concatenate([res.results[i]["out"] for i in range(N_CORES)],
                          axis=0)
